# revision 1
# baseline (speedup 1.0000x reference)
"""Trainium2 kernel for nn_DirectForce (gnn_message_passing).

Math (see reference):
    h   = softplus(X @ w1 + b1) - log(2)          per-edge MLP        [E, 64]
    mag = h @ w2 + b2                                                  [E, 1]
    mag = mag - mean_over_center(mag)[center]      scatter-mean debias
    pair-average mag between each directed edge and its reverse edge
    F   = segment_sum(mag * unit_vec, center)                          [N, 3]

The pair keys (center+neigh+length+|unit|) are identical exactly for the two
directions of each undirected edge (reverse edge has negated vector, same
length), so the sorted-pair averaging pairs each edge with its reverse.  Since
unit_rev = -unit, the pair-averaged scatter reduces algebraically to
    F = segsum(0.5*mag*unit, center) - segsum(0.5*mag*unit, neigh)
which removes the argsort entirely (verified to 2.5e-8 vs the reference).

Device (8 NeuronCores, SPMD, edges partitioned contiguously 200k/core):
  - features pre-transposed AND pre-tiled on host to [NTILES, 128, XT_TILE]
    per core so every input DMA tile is contiguous in HBM (measured 395 GB/s
    vs ~330 GB/s for the strided [128, E_pad] panel layout); tiny MLP weights
    replicated (per the sharding hint)
  - mm1: two float32r matmuls (1 cyc/row) with zero-padded weights [w1;0] and
    [0;w1] accumulate stacked z = [zA; zB] [128, 512] in PSUM per 1024 edges
  - softplus: ACT Exp(z + b1) then Ln(e + 1); the activation-table patch pins
    Exp+Ln to the one table set containing both (otherwise bacc reloads the
    ACT table before every op, 1.3us each)
  - mm2: one matmul per superchunk: lhsT = w2 block-diag [128,2] (2-col LDW is
    free), rhs = h [128, 512] -> mag [2, 512] (row 0 = chunk A, row 1 = B)
  - DVE copies mag [2,512] PSUM->SBUF staging, DMA out per 4 superchunks
Host post (index-structured tail, ~6% of input bytes, numpy):
  - debias via bincount, unit vectors, the two segment sums above.
"""

import os

import numpy as np

N_ATOMS = 50000
E_TOT = 1600000
D_FEAT = 128
H_DIM = 64
N_CORES = 8
EC = E_TOT // N_CORES          # 200000 edges per core
SC = 1024                      # edges per superchunk (2 chunks of 512)
NSC = (EC + SC - 1) // SC      # 196 superchunks
ECP = NSC * SC                 # 200704 padded edges per core
XT_TILE = 8192                 # edges per input DMA (4 MiB)
MAG_GRP = 4                    # superchunks per mag staging DMA
NGRP = NSC // MAG_GRP          # 49

USE_F32R = os.environ.get("KERNEL_F32R", "1") == "1"

# input-tile taper: small tiles at the head (compute starts fast) and tail
# (short pipeline drain), big 8-superchunk tiles between
TILE_SIZES = [1, 1, 2, 4] + [8] * 22 + [4, 2, 2, 2, 1, 1]
assert sum(TILE_SIZES) == NSC
NTILES = len(TILE_SIZES)

_CACHE = {}
LAST_RESULTS = None


def _patch_act_tables():
    """Make Exp and Ln resolve to the single table set that contains both
    (natural_log_exp_and_others) so the ACT table is loaded exactly once.
    Table-set ids are positional, so keys/order are preserved."""
    import functools
    import concourse.hw_specs as hw_specs
    import concourse.bacc as bacc_mod
    import concourse.mybir as mybir

    if _CACHE.get("tables_patched"):
        return
    orig = hw_specs.get_activation_tables
    Exp = mybir.ActivationFunctionType.Exp
    Ln = mybir.ActivationFunctionType.Ln

    def patched(arch):
        out = {}
        for name, fns in orig(arch).items():
            if name != "natural_log_exp_and_others":
                fns = fns - {Exp, Ln}
            out[name] = fns
        return out

    cached = functools.cache(patched)
    hw_specs.get_activation_tables = cached
    bacc_mod.get_activation_tables = cached
    _CACHE["tables_patched"] = True


def _build_nc():
    import concourse.bacc as bacc
    import concourse.mybir as mybir
    import concourse.tile as tile

    _patch_act_tables()

    F32 = mybir.dt.float32
    F32R = mybir.dt.float32r
    MM = F32R if USE_F32R else F32
    Exp = mybir.ActivationFunctionType.Exp
    Ln = mybir.ActivationFunctionType.Ln

    nc = bacc.Bacc("TRN2", target_bir_lowering=False, debug=False)
    xt_d = nc.dram_tensor("xt", [NTILES, 128, XT_TILE], F32, kind="ExternalInput")
    w1a_d = nc.dram_tensor("w1a", [128, 128], F32, kind="ExternalInput")
    w1b_d = nc.dram_tensor("w1b", [128, 128], F32, kind="ExternalInput")
    b1_d = nc.dram_tensor("b1s", [128, 1], F32, kind="ExternalInput")
    w2_d = nc.dram_tensor("w2d", [128, 2], F32, kind="ExternalInput")
    mag_d = nc.dram_tensor("mag", [NGRP, 2, MAG_GRP * 512], F32, kind="ExternalOutput")

    with tile.TileContext(nc) as tc:
        with (
            tc.tile_pool(name="wp", bufs=1) as wp,
            tc.tile_pool(name="xp", bufs=4) as xp,
            tc.tile_pool(name="ep", bufs=6) as ep,
            tc.tile_pool(name="hp", bufs=6) as hp,
            tc.tile_pool(name="mp", bufs=3) as mp,
            tc.tile_pool(name="zp", bufs=5, space="PSUM") as zp,
            tc.tile_pool(name="magp", bufs=3, space="PSUM") as magp,
        ):
            w1a = wp.tile([128, 128], F32, tag="w1a")
            w1b = wp.tile([128, 128], F32, tag="w1b")
            b1s = wp.tile([128, 1], F32, tag="b1s")
            w2d = wp.tile([128, 2], F32, tag="w2d")
            nc.gpsimd.dma_start(w1a[:], w1a_d[:])
            nc.gpsimd.dma_start(w1b[:], w1b_d[:])
            nc.gpsimd.dma_start(b1s[:], b1_d[:])
            nc.gpsimd.dma_start(w2d[:], w2_d[:])
            if USE_F32R:
                w1a_m = wp.tile([128, 128], MM, tag="w1a_r")
                w1b_m = wp.tile([128, 128], MM, tag="w1b_r")
                w2d_m = wp.tile([128, 2], MM, tag="w2d_r")
                nc.vector.tensor_copy(w1a_m[:], w1a[:])
                nc.vector.tensor_copy(w1b_m[:], w1b[:])
                nc.vector.tensor_copy(w2d_m[:], w2d[:])
            else:
                w1a_m, w1b_m, w2d_m = w1a, w1b, w2d

            mag_sb = None
            g = 0
            for ti, size in enumerate(TILE_SIZES):
                width = size * SC
                xt = xp.tile([128, XT_TILE], MM, tag="xt")
                src = xt_d[ti, :, :width]
                nc.sync.dma_start(
                    xt[:, :width], src.bitcast(MM) if USE_F32R else src
                )
                for t in range(size):
                    off = t * SC
                    z_ps = zp.tile([128, 512], F32, tag="z")
                    nc.tensor.matmul(
                        z_ps[:], w1a_m[:], xt[:, off:off + 512],
                        start=True, stop=False,
                    )
                    nc.tensor.matmul(
                        z_ps[:], w1b_m[:], xt[:, off + 512:off + 1024],
                        start=False, stop=True,
                    )
                    # Exp per superchunk (PSUM src); Ln merged over pairs when
                    # the tile is wide enough (amortizes ACT per-op overhead)
                    ei = g % 2
                    if ei == 0:
                        e_sb = ep.tile([128, 1024], F32, tag="e")
                        h_sb = hp.tile([128, 1024], MM, tag="h")
                    nc.scalar.activation(
                        e_sb[:, ei * 512:(ei + 1) * 512], z_ps[:],
                        Exp, bias=b1s[:, :1],
                    )
                    if ei == 1:
                        nc.scalar.activation(h_sb[:], e_sb[:], Ln, bias=1.0)
                        for s2 in range(2):
                            mag_ps = magp.tile([2, 512], F32, tag="mag")
                            nc.tensor.matmul(
                                mag_ps[:], w2d_m[:],
                                h_sb[:, s2 * 512:(s2 + 1) * 512],
                                start=True, stop=True,
                            )
                            gg = g - 1 + s2
                            gi = gg % MAG_GRP
                            if gi == 0:
                                mag_sb = mp.tile([2, MAG_GRP * 512], F32, tag="magsb")
                            nc.vector.tensor_copy(
                                mag_sb[:, gi * 512:(gi + 1) * 512], mag_ps[:]
                            )
                            if gi == MAG_GRP - 1:
                                nc.gpsimd.dma_start(mag_d[gg // MAG_GRP], mag_sb[:])
                    g += 1
    nc.compile()
    return nc


def _get_nc():
    if "nc" not in _CACHE:
        _CACHE["nc"] = _build_nc()
    return _CACHE["nc"]


def kernel(features, edge_vectors, edge_lengths, edge_index, w1, b1, w2, b2):
    global LAST_RESULTS
    from concourse.bass_utils import run_bass_kernel_spmd

    features = np.asarray(features, dtype=np.float32)
    edge_vectors = np.asarray(edge_vectors, dtype=np.float32)
    edge_lengths = np.asarray(edge_lengths, dtype=np.float32)
    edge_index = np.asarray(edge_index)
    w1 = np.asarray(w1, dtype=np.float32)
    b1 = np.asarray(b1, dtype=np.float32).reshape(-1)
    w2 = np.asarray(w2, dtype=np.float32).reshape(-1, 1)
    b2 = np.asarray(b2, dtype=np.float32).reshape(-1)

    # replicated small weights, padded for the stacked-z / block-diag tricks
    w1a = np.zeros((128, 128), np.float32)
    w1a[:, :H_DIM] = w1
    w1b = np.zeros((128, 128), np.float32)
    w1b[:, H_DIM:] = w1
    b1s = np.concatenate([b1, b1]).astype(np.float32).reshape(128, 1)
    w2d = np.zeros((128, 2), np.float32)
    w2d[:H_DIM, 0] = w2[:, 0]
    w2d[H_DIM:, 1] = w2[:, 0]

    # shard edges contiguously across cores; per-core transposed feature panel
    in_maps = []
    for c in range(N_CORES):
        sl = slice(c * EC, (c + 1) * EC)
        panel = np.zeros((128, ECP), np.float32)
        panel[:, :EC] = features[sl].T
        xt = np.zeros((NTILES, 128, XT_TILE), np.float32)
        a = 0
        for ti, size in enumerate(TILE_SIZES):
            w = size * SC
            xt[ti, :, :w] = panel[:, a:a + w]
            a += w
        in_maps.append({"xt": xt, "w1a": w1a, "w1b": w1b, "b1s": b1s, "w2d": w2d})

    nc = _get_nc()
    try:
        res = run_bass_kernel_spmd(nc, in_maps, core_ids=list(range(N_CORES)))
    except Exception:
        # one retry for transient runtime failures
        import time
        time.sleep(2.0)
        res = run_bass_kernel_spmd(nc, in_maps, core_ids=list(range(N_CORES)))
    LAST_RESULTS = res

    # decode mag: out [NGRP, 2, MAG_GRP*512]; within group: col = s*512 + c,
    # value = edge g*4096 + row*512 + s*1024 + c
    mag = np.empty(E_TOT, np.float32)
    for c in range(N_CORES):
        arr = res.results[c]["mag"]  # [NGRP, 2, 2048]
        mc = arr.reshape(NGRP, 2, MAG_GRP, 512).transpose(0, 2, 1, 3).reshape(-1)
        mag[c * EC:(c + 1) * EC] = mc[:EC]

    # fold b2 and the shifted-softplus constant: h_ref = h_dev - log(2)
    mag = mag + (b2[0] - np.float32(np.log(2.0)) * w2.sum())

    center = edge_index[0].astype(np.int64)
    neigh = edge_index[1].astype(np.int64)

    # scatter-mean debias per center atom
    cnt = np.bincount(center, minlength=N_ATOMS).astype(np.float32)
    ssum = np.bincount(center, weights=mag.astype(np.float64), minlength=N_ATOMS)
    bias = (ssum / np.maximum(cnt, 1.0)).astype(np.float32)
    mag = mag - bias[center]

    # pair-averaged antisymmetric force assembly (see module docstring)
    unit = edge_vectors / edge_lengths[:, None]
    val = (0.5 * mag)[:, None] * unit  # [E, 3]
    forces = np.zeros((N_ATOMS, 3), np.float32)
    for k in range(3):
        fc = np.bincount(center, weights=val[:, k].astype(np.float64), minlength=N_ATOMS)
        fn = np.bincount(neigh, weights=val[:, k].astype(np.float64), minlength=N_ATOMS)
        forces[:, k] = (fc - fn).astype(np.float32)
    return forces



# revision 13
# speedup vs baseline: 1.0353x; 1.0353x over previous
"""Trainium2 kernel for nn_DirectForce (gnn_message_passing).

Math (see reference):
    h   = softplus(X @ w1 + b1) - log(2)          per-edge MLP        [E, 64]
    mag = h @ w2 + b2                                                  [E, 1]
    mag = mag - mean_over_center(mag)[center]      scatter-mean debias
    pair-average mag between each directed edge and its reverse edge
    F   = segment_sum(mag * unit_vec, center)                          [N, 3]

The pair keys (center+neigh+length+|unit|) are identical exactly for the two
directions of each undirected edge (reverse edge has negated vector, same
length), so the sorted-pair averaging pairs each edge with its reverse.  Since
unit_rev = -unit, the pair-averaged scatter reduces algebraically to
    F = segsum(0.5*mag*unit, center) - segsum(0.5*mag*unit, neigh)
which removes the argsort entirely (verified to 2.5e-8 vs the reference).

Device (8 NeuronCores, SPMD, edges partitioned contiguously 200k/core):
  - features pre-transposed, cast to fp16, and pre-tiled on host to
    [NTILES, 128, XT_TILE] per core so every input DMA tile is contiguous in
    HBM; fp16 halves the dominant HBM traffic (819MB -> 410MB total).
  - mm1 in fp16: two matmuls per superchunk with zero-padded weights [w1;0]
    and [0;w1] accumulate stacked z = [zA; zB] [128, 512] f32 in PSUM.
    fp16 weights use the separate-LDWEIGHTS path, which the PE overlaps with
    in-flight matmuls via the background weight buffer (f32r could not:
    self-loading matmul serializes LDW+MM, which was the old 367us wall).
  - softplus: two ACT passes (this act_info has no Softplus table): Exp
    (bias=b1) per [128, 1024] z pair (two adjacent PSUM banks) into an fp16
    e tile, then ONE wide Ln(x+1) per 8-superchunk group over [128, 4096].
    Wide ops amortize the ~352-cycle ACT per-op overhead (old: 270us busy,
    new: ~0.99 ns/edge ~= 200us).  ACT is the remaining bottleneck; exp/ln
    have no single-table replacement and no other engine can evaluate them.
  - mm2: per h half, one matmul with a [128, 16] fp16 block-diag w2 variant
    (nonzero cols 2v:2v+2) accumulating into a shared [16, 512] f32 PSUM
    bank across 8 superchunks; rows it doesn't own get +0.  The full bank
    takes ONE batched DVE copy + DMA per 8192 edges (kills the old 149us of
    per-pair DVE copies; DMA itself has no PSUM port on TRN2), and the row
    layout is chosen so host decode is a single flat reshape.
Host post (index-structured tail, ~6% of input bytes, numpy):
  - debias via bincount, unit vectors, the two segment sums above.
"""

import numpy as np

N_ATOMS = 50000
E_TOT = 1600000
D_FEAT = 128
H_DIM = 64
N_CORES = 8
EC = E_TOT // N_CORES          # 200000 edges per core
SC = 1024                      # edges per superchunk (2 chunks of 512)
NSC = (EC + SC - 1) // SC      # 196 superchunks
ECP = NSC * SC                 # 200704 padded edges per core
XT_TILE = 8192                 # edges per input DMA tile
GRP = 8                        # superchunks per mag PSUM bank (16 rows)
NMG = (NSC + GRP - 1) // GRP   # 25 mag banks (last one half-filled)

# input-tile taper: small tiles at the head (compute starts fast) and tail
# (short pipeline drain), big 8-superchunk tiles between
TILE_SIZES = [1, 1, 2, 4] + [8] * 22 + [4, 2, 2, 2, 1, 1]
assert sum(TILE_SIZES) == NSC
NTILES = len(TILE_SIZES)

_CACHE = {}
LAST_RESULTS = None


def _build_nc():
    import concourse.bacc as bacc
    import concourse.mybir as mybir
    import concourse.tile as tile

    F32 = mybir.dt.float32
    F16 = mybir.dt.float16
    Exp = mybir.ActivationFunctionType.Exp
    Ln = mybir.ActivationFunctionType.Ln

    nc = bacc.Bacc("TRN2", target_bir_lowering=False, debug=False)
    xt_d = nc.dram_tensor("xt", [NTILES, 128, XT_TILE], F16, kind="ExternalInput")
    w1a_d = nc.dram_tensor("w1a", [128, 128], F16, kind="ExternalInput")
    w1b_d = nc.dram_tensor("w1b", [128, 128], F16, kind="ExternalInput")
    b1_d = nc.dram_tensor("b1s", [128, 1], F32, kind="ExternalInput")
    w2v_d = nc.dram_tensor("w2v", [128, GRP * 16], F16, kind="ExternalInput")
    mag_d = nc.dram_tensor("mag", [NMG, 16, 512], F32, kind="ExternalOutput")

    with tile.TileContext(nc) as tc:
        with (
            tc.tile_pool(name="wp", bufs=1) as wp,
            tc.tile_pool(name="xp", bufs=4) as xp,
            tc.tile_pool(name="ep", bufs=2) as ep,
            tc.tile_pool(name="hp", bufs=2) as hp,
            tc.tile_pool(name="mp", bufs=3) as mp,
            tc.tile_pool(name="zp", bufs=3, space="PSUM") as zp,
            tc.tile_pool(name="magp", bufs=2, space="PSUM") as magp,
        ):
            w1a = wp.tile([128, 128], F16, tag="w1a")
            w1b = wp.tile([128, 128], F16, tag="w1b")
            b1s = wp.tile([128, 1], F32, tag="b1s")
            w2v = wp.tile([128, GRP * 16], F16, tag="w2v")
            nc.gpsimd.dma_start(w1a[:], w1a_d[:])
            nc.gpsimd.dma_start(w1b[:], w1b_d[:])
            nc.gpsimd.dma_start(b1s[:], b1_d[:])
            nc.gpsimd.dma_start(w2v[:], w2v_d[:])

            z_ps = None
            e_sb = None
            pending_mm2 = []   # [(h tile, n_halves, mag group idx)]

            def emit_mm2():
                # mm2 batch for a finished group, emitted AFTER the next
                # pair's mm1s so the PE's strict FIFO never stalls on Ln
                h_sb, nh, gi = pending_mm2.pop()
                mag_ps = magp.tile([16, 512], F32, tag="mag")
                for v in range(nh):
                    nc.tensor.matmul(
                        mag_ps[:], w2v[:, 16 * v:16 * (v + 1)],
                        h_sb[:, v * 512:(v + 1) * 512],
                        start=(v == 0), stop=(v == nh - 1),
                        skip_group_check=True,
                    )
                # DMA has no PSUM port on TRN2: one batched DVE copy per
                # bank, then DMA from SBUF
                mag_sb = mp.tile([16, 512], F32, tag="magsb")
                nc.vector.tensor_copy(mag_sb[:], mag_ps[:])
                nc.gpsimd.dma_start(mag_d[gi], mag_sb[:])

            g = 0
            for ti, size in enumerate(TILE_SIZES):
                width = size * SC
                xt = xp.tile([128, XT_TILE], F16, tag="xt")
                nc.sync.dma_start(xt[:, :width], xt_d[ti, :, :width])
                for t in range(size):
                    off = t * SC
                    j = g % GRP          # superchunk slot within the group
                    ei = g % 2           # slot within the z pair tile
                    if ei == 0:
                        z_ps = zp.tile([128, 1024], F32, tag="z")
                    zs = z_ps[:, ei * 512:(ei + 1) * 512]
                    nc.tensor.matmul(
                        zs, w1a[:], xt[:, off:off + 512],
                        start=True, stop=False,
                    )
                    nc.tensor.matmul(
                        zs, w1b[:], xt[:, off + 512:off + 1024],
                        start=False, stop=True,
                    )
                    if ei == 1:
                        if pending_mm2:
                            emit_mm2()
                        q = (j - 1) // 2     # pair slot within the group
                        if q == 0:
                            e_sb = ep.tile([128, GRP * 512], F16, tag="e")
                        nc.scalar.activation(
                            e_sb[:, q * 1024:(q + 1) * 1024], z_ps[:],
                            Exp, bias=b1s[:, :1],
                        )
                        if j == GRP - 1 or g + 1 == NSC:
                            w = (q + 1) * 1024
                            h_sb = hp.tile([128, GRP * 512], F16, tag="h")
                            nc.scalar.activation(
                                h_sb[:, :w], e_sb[:, :w], Ln, bias=1.0,
                            )
                            pending_mm2.append((h_sb, 2 * (q + 1), g // GRP))
                    g += 1
            while pending_mm2:
                emit_mm2()
    nc.compile()
    return nc


def _get_nc():
    if "nc" not in _CACHE:
        _CACHE["nc"] = _build_nc()
    return _CACHE["nc"]


def kernel(features, edge_vectors, edge_lengths, edge_index, w1, b1, w2, b2):
    global LAST_RESULTS
    from concourse.bass_utils import run_bass_kernel_spmd

    features = np.asarray(features, dtype=np.float32)
    edge_vectors = np.asarray(edge_vectors, dtype=np.float32)
    edge_lengths = np.asarray(edge_lengths, dtype=np.float32)
    edge_index = np.asarray(edge_index)
    w1 = np.asarray(w1, dtype=np.float32)
    b1 = np.asarray(b1, dtype=np.float32).reshape(-1)
    w2 = np.asarray(w2, dtype=np.float32).reshape(-1, 1)
    b2 = np.asarray(b2, dtype=np.float32).reshape(-1)

    # replicated small weights, padded for the stacked-z / block-diag tricks
    w1a = np.zeros((128, 128), np.float16)
    w1a[:, :H_DIM] = w1
    w1b = np.zeros((128, 128), np.float16)
    w1b[:, H_DIM:] = w1
    b1s = np.concatenate([b1, b1]).astype(np.float32).reshape(128, 1)
    # mm2 variant v (cols 16v:16v+16) owns mag-bank rows 2v:2v+2: within the
    # variant, col 2v+r contracts w2 against the r-th 64-partition half of h
    w2v = np.zeros((128, GRP * 16), np.float16)
    for v in range(GRP):
        w2v[:H_DIM, 16 * v + 2 * v] = w2[:, 0]
        w2v[H_DIM:, 16 * v + 2 * v + 1] = w2[:, 0]

    # shard edges contiguously across cores; per-core transposed fp16 panel
    in_maps = []
    for c in range(N_CORES):
        sl = slice(c * EC, (c + 1) * EC)
        panel = np.zeros((128, ECP), np.float16)
        panel[:, :EC] = features[sl].T
        xt = np.zeros((NTILES, 128, XT_TILE), np.float16)
        a = 0
        for ti, size in enumerate(TILE_SIZES):
            w = size * SC
            xt[ti, :, :w] = panel[:, a:a + w]
            a += w
        in_maps.append({"xt": xt, "w1a": w1a, "w1b": w1b, "b1s": b1s, "w2v": w2v})

    nc = _get_nc()
    try:
        res = run_bass_kernel_spmd(nc, in_maps, core_ids=list(range(N_CORES)))
    except Exception:
        # one retry for transient runtime failures
        import time
        time.sleep(2.0)
        res = run_bass_kernel_spmd(nc, in_maps, core_ids=list(range(N_CORES)))
    LAST_RESULTS = res

    # decode mag: out [NMG, 16, 512]; flat index (G*8 + j)*1024 + r*512 + c
    # equals the edge index directly, so decode is a flat reshape
    mag = np.empty(E_TOT, np.float32)
    for c in range(N_CORES):
        arr = res.results[c]["mag"]  # [NMG, 16, 512]
        mag[c * EC:(c + 1) * EC] = arr.reshape(-1)[:EC]

    # fold b2 and the shifted-softplus constant: h_ref = h_dev - log(2)
    mag = mag + (b2[0] - np.float32(np.log(2.0)) * w2.sum())

    center = edge_index[0].astype(np.int64)
    neigh = edge_index[1].astype(np.int64)

    # scatter-mean debias per center atom
    cnt = np.bincount(center, minlength=N_ATOMS).astype(np.float32)
    ssum = np.bincount(center, weights=mag.astype(np.float64), minlength=N_ATOMS)
    bias = (ssum / np.maximum(cnt, 1.0)).astype(np.float32)
    mag = mag - bias[center]

    # pair-averaged antisymmetric force assembly (see module docstring)
    unit = edge_vectors / edge_lengths[:, None]
    val = (0.5 * mag)[:, None] * unit  # [E, 3]
    forces = np.zeros((N_ATOMS, 3), np.float32)
    for k in range(3):
        fc = np.bincount(center, weights=val[:, k].astype(np.float64), minlength=N_ATOMS)
        fn = np.bincount(neigh, weights=val[:, k].astype(np.float64), minlength=N_ATOMS)
        forces[:, k] = (fc - fn).astype(np.float32)
    return forces


# revision 21
# speedup vs baseline: 1.3976x; 1.3499x over previous
"""Trainium2 kernel for nn_DirectForce (gnn_message_passing).

Math (see reference):
    h   = softplus(X @ w1 + b1) - log(2)          per-edge MLP        [E, 64]
    mag = h @ w2 + b2                                                  [E, 1]
    mag = mag - mean_over_center(mag)[center]      scatter-mean debias
    pair-average mag between each directed edge and its reverse edge
    F   = segment_sum(mag * unit_vec, center)                          [N, 3]

The pair keys (center+neigh+length+|unit|) are identical exactly for the two
directions of each undirected edge (reverse edge has negated vector, same
length), so the sorted-pair averaging pairs each edge with its reverse.  Since
unit_rev = -unit, the pair-averaged scatter reduces algebraically to
    F = segsum(0.5*mag*unit, center) - segsum(0.5*mag*unit, neigh)
which removes the argsort entirely (verified to 2.5e-8 vs the reference).

Device (8 NeuronCores, SPMD, edges partitioned contiguously 200k/core):
  - features pre-transposed, cast to fp16, and pre-tiled on host to
    [NTILES, 128, XT_TILE] per core so every input DMA tile is contiguous in
    HBM; fp16 halves the dominant HBM traffic (819MB -> 410MB total).
  - mm1 in fp16: two matmuls per superchunk with zero-padded weights [w1;0]
    and [0;w1] accumulate stacked z = [zA; zB] [128, 512] f32 in PSUM.
    fp16 weights use the separate-LDWEIGHTS path, which the PE overlaps with
    in-flight matmuls via the background weight buffer (f32r could not:
    self-loading matmul serializes LDW+MM, which was the old 367us wall).
  - softplus: two ACT passes (this act_info has no Softplus table): Exp
    (bias=b1) per [128, 1024] z pair (two adjacent PSUM banks) into an fp16
    e tile, then ONE wide Ln(x+1) per 8-superchunk group over [128, 4096].
    Wide ops amortize the ~352-cycle ACT per-op overhead (old: 270us busy,
    new: ~0.99 ns/edge ~= 200us).  ACT is the remaining bottleneck; exp/ln
    have no single-table replacement and no other engine can evaluate them.
  - mm2: per h half, one matmul with a [128, 16] fp16 block-diag w2 variant
    (nonzero cols 2v:2v+2) accumulating into a shared [16, 512] f32 PSUM
    bank across 8 superchunks; rows it doesn't own get +0.  The full bank
    takes ONE batched DVE copy + DMA per 8192 edges (kills the old 149us of
    per-pair DVE copies; DMA itself has no PSUM port on TRN2), and the row
    layout is chosen so host decode is a single flat reshape.
Host post (index-structured tail, ~6% of input bytes, numpy):
  - debias via bincount, unit vectors, the two segment sums above.
"""

import numpy as np

N_ATOMS = 50000
E_TOT = 1600000
D_FEAT = 128
H_DIM = 64
N_CORES = 8
EC = E_TOT // N_CORES          # 200000 edges per core
SC = 1024                      # edges per superchunk (2 chunks of 512)
NSC = (EC + SC - 1) // SC      # 196 superchunks
ECP = NSC * SC                 # 200704 padded edges per core
XT_TILE = 8192                 # edges per input DMA tile
GRP = 16                       # superchunks per mag PSUM bank (32 rows)
NMG = (NSC + GRP - 1) // GRP   # 13 mag banks (last one quarter-filled)

# input-tile taper: small tiles at the head (compute starts fast) and tail
# (short pipeline drain), big 8-superchunk tiles between
TILE_SIZES = [1, 1, 2, 4] + [8] * 22 + [4, 2, 2, 2, 1, 1]
assert sum(TILE_SIZES) == NSC
NTILES = len(TILE_SIZES)

_CACHE = {}
LAST_RESULTS = None


def _patch_act_tables():
    """Make Exp and Ln resolve to the single table set that contains both
    (natural_log_exp_and_others) so the ACT table is loaded exactly once;
    the default per-op greedy choice flip-flops between exp_and_friends and
    natural_log, paying ~1.5us per reload.  Table-set ids are positional,
    so keys/order are preserved."""
    import functools
    import concourse.hw_specs as hw_specs
    import concourse.bacc as bacc_mod
    import concourse.mybir as mybir

    if _CACHE.get("tables_patched"):
        return
    orig = hw_specs.get_activation_tables
    Exp = mybir.ActivationFunctionType.Exp
    Ln = mybir.ActivationFunctionType.Ln

    def patched(arch):
        out = {}
        for name, fns in orig(arch).items():
            if name != "natural_log_exp_and_others":
                fns = fns - {Exp, Ln}
            out[name] = fns
        return out

    cached = functools.cache(patched)
    hw_specs.get_activation_tables = cached
    bacc_mod.get_activation_tables = cached
    _CACHE["tables_patched"] = True


def _build_nc():
    import concourse.bacc as bacc
    import concourse.mybir as mybir
    import concourse.tile as tile

    _patch_act_tables()

    F32 = mybir.dt.float32
    F16 = mybir.dt.float16
    Exp = mybir.ActivationFunctionType.Exp
    Ln = mybir.ActivationFunctionType.Ln

    nc = bacc.Bacc("TRN2", target_bir_lowering=False, debug=False)
    xt_d = nc.dram_tensor("xt", [NTILES, 128, XT_TILE], F16, kind="ExternalInput")
    w1a_d = nc.dram_tensor("w1a", [128, 128], F16, kind="ExternalInput")
    w1b_d = nc.dram_tensor("w1b", [128, 128], F16, kind="ExternalInput")
    b1_d = nc.dram_tensor("b1s", [128, 1], F32, kind="ExternalInput")
    w2v_d = nc.dram_tensor("w2v", [128, GRP * 32], F16, kind="ExternalInput")
    mag_d = nc.dram_tensor("mag", [NMG, 32, 512], F32, kind="ExternalOutput")

    with tile.TileContext(nc) as tc:
        with (
            tc.tile_pool(name="wp", bufs=1) as wp,
            tc.tile_pool(name="xp", bufs=4) as xp,
            tc.tile_pool(name="ep", bufs=2) as ep,
            tc.tile_pool(name="hp", bufs=2) as hp,
            tc.tile_pool(name="mp", bufs=3) as mp,
            tc.tile_pool(name="zp", bufs=3, space="PSUM") as zp,
            tc.tile_pool(name="magp", bufs=2, space="PSUM") as magp,
        ):
            w1a = wp.tile([128, 128], F16, tag="w1a")
            w1b = wp.tile([128, 128], F16, tag="w1b")
            b1s = wp.tile([128, 1], F32, tag="b1s")
            w2v = wp.tile([128, GRP * 32], F16, tag="w2v")
            nc.gpsimd.dma_start(w1a[:], w1a_d[:])
            nc.gpsimd.dma_start(w1b[:], w1b_d[:])
            nc.gpsimd.dma_start(b1s[:], b1_d[:])
            nc.gpsimd.dma_start(w2v[:], w2v_d[:])

            z_ps = None
            e_sb = None
            pending_mm2 = []   # [(h tile, n_halves, mag group idx)]

            def emit_mm2():
                # mm2 batch for a finished group, emitted AFTER the next
                # pair's mm1s so the PE's strict FIFO never stalls on Ln
                h_sb, nh, gi = pending_mm2.pop()
                mag_ps = magp.tile([32, 512], F32, tag="mag")
                for v in range(nh):
                    nc.tensor.matmul(
                        mag_ps[:], w2v[:, 32 * v:32 * (v + 1)],
                        h_sb[:, v * 512:(v + 1) * 512],
                        start=(v == 0), stop=(v == nh - 1),
                        skip_group_check=True,
                    )
                # DMA has no PSUM port on TRN2: one batched DVE copy per
                # bank, then DMA from SBUF
                mag_sb = mp.tile([32, 512], F32, tag="magsb")
                nc.vector.tensor_copy(mag_sb[:], mag_ps[:])
                nc.gpsimd.dma_start(mag_d[gi], mag_sb[:])

            g = 0
            for ti, size in enumerate(TILE_SIZES):
                width = size * SC
                xt = xp.tile([128, XT_TILE], F16, tag="xt")
                nc.sync.dma_start(xt[:, :width], xt_d[ti, :, :width])
                for t in range(size):
                    off = t * SC
                    j = g % GRP          # superchunk slot within the group
                    ei = g % 2           # slot within the z pair tile
                    if ei == 0:
                        z_ps = zp.tile([128, 1024], F32, tag="z")
                    zs = z_ps[:, ei * 512:(ei + 1) * 512]
                    nc.tensor.matmul(
                        zs, w1a[:], xt[:, off:off + 512],
                        start=True, stop=False,
                    )
                    nc.tensor.matmul(
                        zs, w1b[:], xt[:, off + 512:off + 1024],
                        start=False, stop=True,
                    )
                    if ei == 1:
                        if pending_mm2:
                            emit_mm2()
                        q = (j - 1) // 2     # pair slot within the group
                        if q == 0:
                            e_sb = ep.tile([128, GRP * 512], F16, tag="e")
                        nc.scalar.activation(
                            e_sb[:, q * 1024:(q + 1) * 1024], z_ps[:],
                            Exp, bias=b1s[:, :1],
                        )
                        if j == GRP - 1 or g + 1 == NSC:
                            w = (q + 1) * 1024
                            h_sb = hp.tile([128, GRP * 512], F16, tag="h")
                            nc.scalar.activation(
                                h_sb[:, :w], e_sb[:, :w], Ln, bias=1.0,
                            )
                            pending_mm2.append((h_sb, 2 * (q + 1), g // GRP))
                    g += 1
            while pending_mm2:
                emit_mm2()
    nc.compile()
    return nc


def _get_nc():
    if "nc" not in _CACHE:
        _CACHE["nc"] = _build_nc()
    return _CACHE["nc"]


def kernel(features, edge_vectors, edge_lengths, edge_index, w1, b1, w2, b2):
    global LAST_RESULTS
    from concourse.bass_utils import run_bass_kernel_spmd

    features = np.asarray(features, dtype=np.float32)
    edge_vectors = np.asarray(edge_vectors, dtype=np.float32)
    edge_lengths = np.asarray(edge_lengths, dtype=np.float32)
    edge_index = np.asarray(edge_index)
    w1 = np.asarray(w1, dtype=np.float32)
    b1 = np.asarray(b1, dtype=np.float32).reshape(-1)
    w2 = np.asarray(w2, dtype=np.float32).reshape(-1, 1)
    b2 = np.asarray(b2, dtype=np.float32).reshape(-1)

    # replicated small weights, padded for the stacked-z / block-diag tricks
    w1a = np.zeros((128, 128), np.float16)
    w1a[:, :H_DIM] = w1
    w1b = np.zeros((128, 128), np.float16)
    w1b[:, H_DIM:] = w1
    b1s = np.concatenate([b1, b1]).astype(np.float32).reshape(128, 1)
    # mm2 variant v (cols 32v:32v+32) owns mag-bank rows 2v:2v+2: within the
    # variant, col 2v+r contracts w2 against the r-th 64-partition half of h
    w2v = np.zeros((128, GRP * 32), np.float16)
    for v in range(GRP):
        w2v[:H_DIM, 32 * v + 2 * v] = w2[:, 0]
        w2v[H_DIM:, 32 * v + 2 * v + 1] = w2[:, 0]

    # shard edges contiguously across cores; per-core transposed fp16 panel
    in_maps = []
    for c in range(N_CORES):
        sl = slice(c * EC, (c + 1) * EC)
        panel = np.zeros((128, ECP), np.float16)
        panel[:, :EC] = features[sl].T
        xt = np.zeros((NTILES, 128, XT_TILE), np.float16)
        a = 0
        for ti, size in enumerate(TILE_SIZES):
            w = size * SC
            xt[ti, :, :w] = panel[:, a:a + w]
            a += w
        in_maps.append({"xt": xt, "w1a": w1a, "w1b": w1b, "b1s": b1s, "w2v": w2v})

    nc = _get_nc()
    try:
        res = run_bass_kernel_spmd(nc, in_maps, core_ids=list(range(N_CORES)))
    except Exception:
        # one retry for transient runtime failures
        import time
        time.sleep(2.0)
        res = run_bass_kernel_spmd(nc, in_maps, core_ids=list(range(N_CORES)))
    LAST_RESULTS = res

    # decode mag: out [NMG, 32, 512]; flat index (G*16 + j)*1024 + r*512 + c
    # equals the edge index directly, so decode is a flat reshape
    mag = np.empty(E_TOT, np.float32)
    for c in range(N_CORES):
        arr = res.results[c]["mag"]  # [NMG, 32, 512]
        mag[c * EC:(c + 1) * EC] = arr.reshape(-1)[:EC]

    # fold b2 and the shifted-softplus constant: h_ref = h_dev - log(2)
    mag = mag + (b2[0] - np.float32(np.log(2.0)) * w2.sum())

    center = edge_index[0].astype(np.int64)
    neigh = edge_index[1].astype(np.int64)

    # scatter-mean debias per center atom
    cnt = np.bincount(center, minlength=N_ATOMS).astype(np.float32)
    ssum = np.bincount(center, weights=mag.astype(np.float64), minlength=N_ATOMS)
    bias = (ssum / np.maximum(cnt, 1.0)).astype(np.float32)
    mag = mag - bias[center]

    # pair-averaged antisymmetric force assembly (see module docstring)
    unit = edge_vectors / edge_lengths[:, None]
    val = (0.5 * mag)[:, None] * unit  # [E, 3]
    forces = np.zeros((N_ATOMS, 3), np.float32)
    for k in range(3):
        fc = np.bincount(center, weights=val[:, k].astype(np.float64), minlength=N_ATOMS)
        fn = np.bincount(neigh, weights=val[:, k].astype(np.float64), minlength=N_ATOMS)
        forces[:, k] = (fc - fn).astype(np.float32)
    return forces


# revision 28
# speedup vs baseline: 1.5386x; 1.1009x over previous
"""Trainium2 kernel for nn_DirectForce (gnn_message_passing).

Math (see reference):
    h   = softplus(X @ w1 + b1) - log(2)          per-edge MLP        [E, 64]
    mag = h @ w2 + b2                                                  [E, 1]
    mag = mag - mean_over_center(mag)[center]      scatter-mean debias
    pair-average mag between each directed edge and its reverse edge
    F   = segment_sum(mag * unit_vec, center)                          [N, 3]

The pair keys (center+neigh+length+|unit|) are identical exactly for the two
directions of each undirected edge (reverse edge has negated vector, same
length), so the sorted-pair averaging pairs each edge with its reverse.  Since
unit_rev = -unit, the pair-averaged scatter reduces algebraically to
    F = segsum(0.5*mag*unit, center) - segsum(0.5*mag*unit, neigh)
which removes the argsort entirely (verified to 2.5e-8 vs the reference).

Device (8 NeuronCores, SPMD, edges partitioned contiguously 200k/core):
  - features pre-transposed, cast to fp16, and pre-tiled on host to
    [NTILES, 128, XT_TILE] per core so every input DMA tile is contiguous in
    HBM; fp16 halves the dominant HBM traffic (819MB -> 410MB total).
  - mm1 in fp16: two matmuls per superchunk with zero-padded weights [w1;0]
    and [0;w1] accumulate stacked z = [zA; zB] [128, 512] f32 in PSUM.
    fp16 weights use the separate-LDWEIGHTS path, which the PE overlaps with
    in-flight matmuls via the background weight buffer (f32r could not:
    self-loading matmul serializes LDW+MM, which was the old 367us wall).
  - softplus: two ACT passes (this act_info has no Softplus table): Exp
    (bias=b1) per [128, 1024] z pair (two adjacent PSUM banks) into an fp16
    e tile, then ONE wide Ln(x+1) per 8-superchunk group over [128, 4096].
    Wide ops amortize the ~352-cycle ACT per-op overhead (old: 270us busy,
    new: ~0.99 ns/edge ~= 200us).  ACT is the remaining bottleneck; exp/ln
    have no single-table replacement and no other engine can evaluate them.
  - mm2: per h half, one matmul with a [128, 16] fp16 block-diag w2 variant
    (nonzero cols 2v:2v+2) accumulating into a shared [16, 512] f32 PSUM
    bank across 8 superchunks; rows it doesn't own get +0.  The full bank
    takes ONE batched DVE copy + DMA per 8192 edges (kills the old 149us of
    per-pair DVE copies; DMA itself has no PSUM port on TRN2), and the row
    layout is chosen so host decode is a single flat reshape.
Host post (index-structured tail, ~6% of input bytes, numpy):
  - debias via bincount, unit vectors, the two segment sums above.
"""

import numpy as np

N_ATOMS = 50000
E_TOT = 1600000
D_FEAT = 128
H_DIM = 64
N_CORES = 8
EC = E_TOT // N_CORES          # 200000 edges per core
SC = 1024                      # edges per superchunk (2 chunks of 512)
NSC = (EC + SC - 1) // SC      # 196 superchunks
ECP = NSC * SC                 # 200704 padded edges per core
XT_TILE = 8192                 # edges per input DMA tile
GRP = 16                       # superchunks per mag PSUM bank (32 rows)
NMG = (NSC + GRP - 1) // GRP   # 13 mag banks (last one quarter-filled)

# input-tile taper: small tiles at the head (compute starts fast) and tail
# (short pipeline drain), big 8-superchunk tiles between
TILE_SIZES = [1, 1, 2, 4] + [8] * 22 + [4, 2, 2, 2, 1, 1]
assert sum(TILE_SIZES) == NSC
NTILES = len(TILE_SIZES)

_CACHE = {}
LAST_RESULTS = None


def _patch_act_tables():
    """Make Exp and Ln resolve to the single table set that contains both
    (natural_log_exp_and_others) so the ACT table is loaded exactly once;
    the default per-op greedy choice flip-flops between exp_and_friends and
    natural_log, paying ~1.5us per reload.  Table-set ids are positional,
    so keys/order are preserved."""
    import functools
    import concourse.hw_specs as hw_specs
    import concourse.bacc as bacc_mod
    import concourse.mybir as mybir

    if _CACHE.get("tables_patched"):
        return
    orig = hw_specs.get_activation_tables
    Exp = mybir.ActivationFunctionType.Exp
    Ln = mybir.ActivationFunctionType.Ln

    def patched(arch):
        out = {}
        for name, fns in orig(arch).items():
            if name != "natural_log_exp_and_others":
                fns = fns - {Exp, Ln}
            out[name] = fns
        return out

    cached = functools.cache(patched)
    hw_specs.get_activation_tables = cached
    bacc_mod.get_activation_tables = cached
    _CACHE["tables_patched"] = True


def _build_nc():
    import concourse.bacc as bacc
    import concourse.mybir as mybir
    import concourse.tile as tile

    _patch_act_tables()

    F32 = mybir.dt.float32
    F16 = mybir.dt.float16
    Exp = mybir.ActivationFunctionType.Exp
    Ln = mybir.ActivationFunctionType.Ln

    nc = bacc.Bacc("TRN2", target_bir_lowering=False, debug=False)
    xt_d = nc.dram_tensor("xt", [NTILES, 128, XT_TILE], F16, kind="ExternalInput")
    w1a_d = nc.dram_tensor("w1a", [128, 128], F16, kind="ExternalInput")
    w1b_d = nc.dram_tensor("w1b", [128, 128], F16, kind="ExternalInput")
    b1_d = nc.dram_tensor("b1s", [128, 1], F32, kind="ExternalInput")
    w2v_d = nc.dram_tensor("w2v", [128, GRP * 32], F16, kind="ExternalInput")
    mag_d = nc.dram_tensor("mag", [NMG, 32, 512], F32, kind="ExternalOutput")

    with tile.TileContext(nc) as tc:
        with (
            tc.tile_pool(name="wp", bufs=1) as wp,
            tc.tile_pool(name="xp", bufs=4) as xp,
            tc.tile_pool(name="ep", bufs=2) as ep,
            tc.tile_pool(name="hp", bufs=2) as hp,
            tc.tile_pool(name="mp", bufs=3) as mp,
            tc.tile_pool(name="zp", bufs=3, space="PSUM") as zp,
            tc.tile_pool(name="magp", bufs=2, space="PSUM") as magp,
        ):
            w1a = wp.tile([128, 128], F16, tag="w1a")
            w1b = wp.tile([128, 128], F16, tag="w1b")
            b1s = wp.tile([128, 1], F32, tag="b1s")
            w2v = wp.tile([128, GRP * 32], F16, tag="w2v")
            nc.gpsimd.dma_start(w1a[:], w1a_d[:])
            nc.gpsimd.dma_start(w1b[:], w1b_d[:])
            nc.gpsimd.dma_start(b1s[:], b1_d[:])
            nc.gpsimd.dma_start(w2v[:], w2v_d[:])

            z_ps = None
            e_sb = None
            h_state = [None]
            pending_mm2 = []   # [(h tile, v0, v1, bank_done, mag group idx)]

            mag_state = [None]

            def emit_mm2():
                # mm2 half-batch for a finished Ln half, emitted AFTER the
                # next pair's mm1s so the PE's strict FIFO never stalls on Ln
                h_sb, v0, v1, bank_done, gi = pending_mm2.pop(0)
                if v0 == 0:
                    mag_new = magp.tile([32, 512], F32, tag="mag")
                    mag_state[0] = mag_new
                mag_ps = mag_state[0]
                for v in range(v0, v1):
                    nc.tensor.matmul(
                        mag_ps[:], w2v[:, 32 * v:32 * (v + 1)],
                        h_sb[:, v * 512:(v + 1) * 512],
                        start=(v == 0), stop=(bank_done and v == v1 - 1),
                        skip_group_check=True,
                    )
                if bank_done:
                    # bank complete: DMA has no PSUM port on TRN2 -> one
                    # batched DVE copy per bank, then DMA from SBUF
                    mag_sb = mp.tile([32, 512], F32, tag="magsb")
                    nc.vector.tensor_copy(mag_sb[:], mag_ps[:])
                    nc.gpsimd.dma_start(mag_d[gi], mag_sb[:])

            g = 0
            for ti, size in enumerate(TILE_SIZES):
                width = size * SC
                xt = xp.tile([128, XT_TILE], F16, tag="xt")
                nc.sync.dma_start(xt[:, :width], xt_d[ti, :, :width])
                for t in range(size):
                    off = t * SC
                    j = g % GRP          # superchunk slot within the group
                    ei = g % 2           # slot within the z pair tile
                    if ei == 0:
                        z_ps = zp.tile([128, 1024], F32, tag="z")
                    zs = z_ps[:, ei * 512:(ei + 1) * 512]
                    nc.tensor.matmul(
                        zs, w1a[:], xt[:, off:off + 512],
                        start=True, stop=False,
                    )
                    nc.tensor.matmul(
                        zs, w1b[:], xt[:, off + 512:off + 1024],
                        start=False, stop=True,
                    )
                    if ei == 1:
                        if pending_mm2:
                            emit_mm2()
                        q = (j - 1) // 2     # pair slot within the group
                        if q == 0:
                            e_sb = ep.tile([128, GRP * 512], F16, tag="e")
                        nc.scalar.activation(
                            e_sb[:, q * 1024:(q + 1) * 1024], z_ps[:],
                            Exp, bias=b1s[:, :1],
                        )
                        half = GRP // 4   # pairs per Ln half-op
                        if (q + 1) % half == 0 or g + 1 == NSC:
                            q0 = (q // half) * half   # first pair this half
                            if q0 == 0:
                                h_new = hp.tile([128, GRP * 512], F16, tag="h")
                                h_state[0] = h_new
                            hcur = h_state[0]
                            nc.scalar.activation(
                                hcur[:, q0 * 1024:(q + 1) * 1024],
                                e_sb[:, q0 * 1024:(q + 1) * 1024],
                                Ln, bias=1.0,
                            )
                            bank_done = (j == GRP - 1) or (g + 1 == NSC)
                            pending_mm2.append(
                                (hcur, 2 * q0, 2 * (q + 1), bank_done,
                                 g // GRP)
                            )
                    g += 1
            while pending_mm2:
                emit_mm2()
    nc.compile()
    return nc


def _get_nc():
    if "nc" not in _CACHE:
        _CACHE["nc"] = _build_nc()
    return _CACHE["nc"]


def kernel(features, edge_vectors, edge_lengths, edge_index, w1, b1, w2, b2):
    global LAST_RESULTS
    from concourse.bass_utils import run_bass_kernel_spmd

    features = np.asarray(features, dtype=np.float32)
    edge_vectors = np.asarray(edge_vectors, dtype=np.float32)
    edge_lengths = np.asarray(edge_lengths, dtype=np.float32)
    edge_index = np.asarray(edge_index)
    w1 = np.asarray(w1, dtype=np.float32)
    b1 = np.asarray(b1, dtype=np.float32).reshape(-1)
    w2 = np.asarray(w2, dtype=np.float32).reshape(-1, 1)
    b2 = np.asarray(b2, dtype=np.float32).reshape(-1)

    # replicated small weights, padded for the stacked-z / block-diag tricks
    w1a = np.zeros((128, 128), np.float16)
    w1a[:, :H_DIM] = w1
    w1b = np.zeros((128, 128), np.float16)
    w1b[:, H_DIM:] = w1
    b1s = np.concatenate([b1, b1]).astype(np.float32).reshape(128, 1)
    # mm2 variant v (cols 32v:32v+32) owns mag-bank rows 2v:2v+2: within the
    # variant, col 2v+r contracts w2 against the r-th 64-partition half of h
    w2v = np.zeros((128, GRP * 32), np.float16)
    for v in range(GRP):
        w2v[:H_DIM, 32 * v + 2 * v] = w2[:, 0]
        w2v[H_DIM:, 32 * v + 2 * v + 1] = w2[:, 0]

    # shard edges contiguously across cores; per-core transposed fp16 panel
    in_maps = []
    for c in range(N_CORES):
        sl = slice(c * EC, (c + 1) * EC)
        panel = np.zeros((128, ECP), np.float16)
        panel[:, :EC] = features[sl].T
        xt = np.zeros((NTILES, 128, XT_TILE), np.float16)
        a = 0
        for ti, size in enumerate(TILE_SIZES):
            w = size * SC
            xt[ti, :, :w] = panel[:, a:a + w]
            a += w
        in_maps.append({"xt": xt, "w1a": w1a, "w1b": w1b, "b1s": b1s, "w2v": w2v})

    nc = _get_nc()
    try:
        res = run_bass_kernel_spmd(nc, in_maps, core_ids=list(range(N_CORES)))
    except Exception:
        # one retry for transient runtime failures
        import time
        time.sleep(2.0)
        res = run_bass_kernel_spmd(nc, in_maps, core_ids=list(range(N_CORES)))
    LAST_RESULTS = res

    # decode mag: out [NMG, 32, 512]; flat index (G*16 + j)*1024 + r*512 + c
    # equals the edge index directly, so decode is a flat reshape
    mag = np.empty(E_TOT, np.float32)
    for c in range(N_CORES):
        arr = res.results[c]["mag"]  # [NMG, 32, 512]
        mag[c * EC:(c + 1) * EC] = arr.reshape(-1)[:EC]

    # fold b2 and the shifted-softplus constant: h_ref = h_dev - log(2)
    mag = mag + (b2[0] - np.float32(np.log(2.0)) * w2.sum())

    center = edge_index[0].astype(np.int64)
    neigh = edge_index[1].astype(np.int64)

    # scatter-mean debias per center atom
    cnt = np.bincount(center, minlength=N_ATOMS).astype(np.float32)
    ssum = np.bincount(center, weights=mag.astype(np.float64), minlength=N_ATOMS)
    bias = (ssum / np.maximum(cnt, 1.0)).astype(np.float32)
    mag = mag - bias[center]

    # pair-averaged antisymmetric force assembly (see module docstring)
    unit = edge_vectors / edge_lengths[:, None]
    val = (0.5 * mag)[:, None] * unit  # [E, 3]
    forces = np.zeros((N_ATOMS, 3), np.float32)
    for k in range(3):
        fc = np.bincount(center, weights=val[:, k].astype(np.float64), minlength=N_ATOMS)
        fn = np.bincount(neigh, weights=val[:, k].astype(np.float64), minlength=N_ATOMS)
        forces[:, k] = (fc - fn).astype(np.float32)
    return forces


# revision 33
# speedup vs baseline: 1.5388x; 1.0001x over previous
"""Trainium2 kernel for nn_DirectForce (gnn_message_passing).

Math (see reference):
    h   = softplus(X @ w1 + b1) - log(2)          per-edge MLP        [E, 64]
    mag = h @ w2 + b2                                                  [E, 1]
    mag = mag - mean_over_center(mag)[center]      scatter-mean debias
    pair-average mag between each directed edge and its reverse edge
    F   = segment_sum(mag * unit_vec, center)                          [N, 3]

The pair keys (center+neigh+length+|unit|) are identical exactly for the two
directions of each undirected edge (reverse edge has negated vector, same
length), so the sorted-pair averaging pairs each edge with its reverse.  Since
unit_rev = -unit, the pair-averaged scatter reduces algebraically to
    F = segsum(0.5*mag*unit, center) - segsum(0.5*mag*unit, neigh)
which removes the argsort entirely (verified to 2.5e-8 vs the reference).

Device (8 NeuronCores, SPMD, edges partitioned contiguously 200k/core):
  - features pre-transposed, cast to fp16, and pre-tiled on host to
    [NTILES, 128, XT_TILE] per core so every input DMA tile is contiguous in
    HBM; fp16 halves the dominant HBM traffic (819MB -> 410MB total).
  - mm1 in fp16: two matmuls per superchunk with zero-padded weights [w1;0]
    and [0;w1] accumulate stacked z = [zA; zB] [128, 512] f32 in PSUM.
    fp16 weights use the separate-LDWEIGHTS path, which the PE overlaps with
    in-flight matmuls via the background weight buffer (f32r could not:
    self-loading matmul serializes LDW+MM, which was the old 367us wall).
  - softplus: two ACT passes (this act_info has no Softplus table): Exp
    (bias=b1) per [128, 1024] z pair (two adjacent PSUM banks) into an fp16
    e tile, then ONE wide Ln(x+1) per 8-superchunk group over [128, 4096].
    Wide ops amortize the ~352-cycle ACT per-op overhead (old: 270us busy,
    new: ~0.99 ns/edge ~= 200us).  ACT is the remaining bottleneck; exp/ln
    have no single-table replacement and no other engine can evaluate them.
  - mm2: per h half, one matmul with a [128, 16] fp16 block-diag w2 variant
    (nonzero cols 2v:2v+2) accumulating into a shared [16, 512] f32 PSUM
    bank across 8 superchunks; rows it doesn't own get +0.  The full bank
    takes ONE batched DVE copy + DMA per 8192 edges (kills the old 149us of
    per-pair DVE copies; DMA itself has no PSUM port on TRN2), and the row
    layout is chosen so host decode is a single flat reshape.
Host post (index-structured tail, ~6% of input bytes, numpy):
  - debias via bincount, unit vectors, the two segment sums above.
"""

import numpy as np

N_ATOMS = 50000
E_TOT = 1600000
D_FEAT = 128
H_DIM = 64
N_CORES = 8
EC = E_TOT // N_CORES          # 200000 edges per core
SC = 1024                      # edges per superchunk (2 chunks of 512)
NSC = (EC + SC - 1) // SC      # 196 superchunks
ECP = NSC * SC                 # 200704 padded edges per core
XT_TILE = 8192                 # edges per input DMA tile
GRP = 16                       # superchunks per mag PSUM bank (32 rows)
NMG = (NSC + GRP - 1) // GRP   # 13 mag banks (last one quarter-filled)

# input-tile taper: small tiles at the head (compute starts fast) and tail
# (short pipeline drain), big 8-superchunk tiles between
TILE_SIZES = [1, 1, 2, 4] + [8] * 22 + [4, 2, 2, 2, 1, 1]
assert sum(TILE_SIZES) == NSC
NTILES = len(TILE_SIZES)

_CACHE = {}
LAST_RESULTS = None


def _patch_act_tables():
    """Make Exp and Ln resolve to the single table set that contains both
    (natural_log_exp_and_others) so the ACT table is loaded exactly once;
    the default per-op greedy choice flip-flops between exp_and_friends and
    natural_log, paying ~1.5us per reload.  Table-set ids are positional,
    so keys/order are preserved."""
    import functools
    import concourse.hw_specs as hw_specs
    import concourse.bacc as bacc_mod
    import concourse.mybir as mybir

    if _CACHE.get("tables_patched"):
        return
    orig = hw_specs.get_activation_tables
    Exp = mybir.ActivationFunctionType.Exp
    Ln = mybir.ActivationFunctionType.Ln

    def patched(arch):
        out = {}
        for name, fns in orig(arch).items():
            if name != "natural_log_exp_and_others":
                fns = fns - {Exp, Ln}
            out[name] = fns
        return out

    cached = functools.cache(patched)
    hw_specs.get_activation_tables = cached
    bacc_mod.get_activation_tables = cached
    _CACHE["tables_patched"] = True


def _build_nc():
    import concourse.bacc as bacc
    import concourse.mybir as mybir
    import concourse.tile as tile

    _patch_act_tables()

    F32 = mybir.dt.float32
    F16 = mybir.dt.float16
    Exp = mybir.ActivationFunctionType.Exp
    Ln = mybir.ActivationFunctionType.Ln

    nc = bacc.Bacc("TRN2", target_bir_lowering=False, debug=False)
    xt_d = nc.dram_tensor("xt", [NTILES, 128, XT_TILE], F16, kind="ExternalInput")
    w1a_d = nc.dram_tensor("w1a", [128, 128], F16, kind="ExternalInput")
    w1b_d = nc.dram_tensor("w1b", [128, 128], F16, kind="ExternalInput")
    b1_d = nc.dram_tensor("b1s", [128, 1], F32, kind="ExternalInput")
    w2v_d = nc.dram_tensor("w2v", [128, GRP * 32], F16, kind="ExternalInput")
    mag_d = nc.dram_tensor("mag", [NMG, 32, 512], F32, kind="ExternalOutput")

    with tile.TileContext(nc) as tc:
        with (
            tc.tile_pool(name="wp", bufs=1) as wp,
            tc.tile_pool(name="xp", bufs=4) as xp,
            tc.tile_pool(name="ep", bufs=2) as ep,
            tc.tile_pool(name="hp", bufs=2) as hp,
            tc.tile_pool(name="mp", bufs=3) as mp,
            tc.tile_pool(name="zp", bufs=3, space="PSUM") as zp,
            tc.tile_pool(name="magp", bufs=2, space="PSUM") as magp,
        ):
            # dummy early activation: hoists the ~1.5us ACT table load off
            # the first real activation's dependency chain
            dummy = wp.tile([128, 1], F32, tag="dummy")
            nc.vector.memset(dummy[:], 0.0)
            nc.scalar.activation(dummy[:], dummy[:], Exp, bias=0.0)

            w1a = wp.tile([128, 128], F16, tag="w1a")
            w1b = wp.tile([128, 128], F16, tag="w1b")
            b1s = wp.tile([128, 1], F32, tag="b1s")
            w2v = wp.tile([128, GRP * 32], F16, tag="w2v")
            nc.gpsimd.dma_start(w1a[:], w1a_d[:])
            nc.gpsimd.dma_start(w1b[:], w1b_d[:])
            nc.gpsimd.dma_start(b1s[:], b1_d[:])
            nc.gpsimd.dma_start(w2v[:], w2v_d[:])

            z_ps = None
            e_sb = None
            h_state = [None]
            MM2_DELAY = 3      # pairs between Ln-half and its mm2 emission
            pending_mm2 = []   # [(pair#, h tile, v0, v1, bank_done, grp idx)]

            mag_state = [None]

            def emit_mm2():
                # mm2 half-batch for a finished Ln half, emitted MM2_DELAY
                # pairs later so the PE's strict FIFO has mm1 runway (3 z
                # bufs) to cover the Ln latency before blocking on it
                _, h_sb, v0, v1, bank_done, gi = pending_mm2.pop(0)
                if v0 == 0:
                    mag_new = magp.tile([32, 512], F32, tag="mag")
                    mag_state[0] = mag_new
                mag_ps = mag_state[0]
                for v in range(v0, v1):
                    nc.tensor.matmul(
                        mag_ps[:], w2v[:, 32 * v:32 * (v + 1)],
                        h_sb[:, v * 512:(v + 1) * 512],
                        start=(v == 0), stop=(bank_done and v == v1 - 1),
                        skip_group_check=True,
                    )
                if bank_done:
                    # bank complete: DMA has no PSUM port on TRN2 -> one
                    # batched DVE copy per bank, then DMA from SBUF
                    mag_sb = mp.tile([32, 512], F32, tag="magsb")
                    nc.vector.tensor_copy(mag_sb[:], mag_ps[:])
                    nc.gpsimd.dma_start(mag_d[gi], mag_sb[:])

            g = 0
            for ti, size in enumerate(TILE_SIZES):
                width = size * SC
                xt = xp.tile([128, XT_TILE], F16, tag="xt")
                nc.sync.dma_start(xt[:, :width], xt_d[ti, :, :width])
                for t in range(size):
                    off = t * SC
                    j = g % GRP          # superchunk slot within the group
                    ei = g % 2           # slot within the z pair tile
                    if ei == 0:
                        z_ps = zp.tile([128, 1024], F32, tag="z")
                    zs = z_ps[:, ei * 512:(ei + 1) * 512]
                    nc.tensor.matmul(
                        zs, w1a[:], xt[:, off:off + 512],
                        start=True, stop=False,
                    )
                    nc.tensor.matmul(
                        zs, w1b[:], xt[:, off + 512:off + 1024],
                        start=False, stop=True,
                    )
                    if ei == 1:
                        pair_no = g // 2
                        if pending_mm2 and pair_no - pending_mm2[0][0] >= MM2_DELAY:
                            emit_mm2()
                        q = (j - 1) // 2     # pair slot within the group
                        if q == 0:
                            e_sb = ep.tile([128, GRP * 512], F16, tag="e")
                        nc.scalar.activation(
                            e_sb[:, q * 1024:(q + 1) * 1024], z_ps[:],
                            Exp, bias=b1s[:, :1],
                        )
                        half = GRP // 4   # pairs per Ln half-op
                        if (q + 1) % half == 0 or g + 1 == NSC:
                            q0 = (q // half) * half   # first pair this half
                            if q0 == 0:
                                h_new = hp.tile([128, GRP * 512], F16, tag="h")
                                h_state[0] = h_new
                            hcur = h_state[0]
                            nc.scalar.activation(
                                hcur[:, q0 * 1024:(q + 1) * 1024],
                                e_sb[:, q0 * 1024:(q + 1) * 1024],
                                Ln, bias=1.0,
                            )
                            bank_done = (j == GRP - 1) or (g + 1 == NSC)
                            pending_mm2.append(
                                (g // 2, hcur, 2 * q0, 2 * (q + 1), bank_done,
                                 g // GRP)
                            )
                    g += 1
            while pending_mm2:
                emit_mm2()
    nc.compile()
    return nc


def _get_nc():
    if "nc" not in _CACHE:
        _CACHE["nc"] = _build_nc()
    return _CACHE["nc"]


def kernel(features, edge_vectors, edge_lengths, edge_index, w1, b1, w2, b2):
    global LAST_RESULTS
    from concourse.bass_utils import run_bass_kernel_spmd

    features = np.asarray(features, dtype=np.float32)
    edge_vectors = np.asarray(edge_vectors, dtype=np.float32)
    edge_lengths = np.asarray(edge_lengths, dtype=np.float32)
    edge_index = np.asarray(edge_index)
    w1 = np.asarray(w1, dtype=np.float32)
    b1 = np.asarray(b1, dtype=np.float32).reshape(-1)
    w2 = np.asarray(w2, dtype=np.float32).reshape(-1, 1)
    b2 = np.asarray(b2, dtype=np.float32).reshape(-1)

    # replicated small weights, padded for the stacked-z / block-diag tricks
    w1a = np.zeros((128, 128), np.float16)
    w1a[:, :H_DIM] = w1
    w1b = np.zeros((128, 128), np.float16)
    w1b[:, H_DIM:] = w1
    b1s = np.concatenate([b1, b1]).astype(np.float32).reshape(128, 1)
    # mm2 variant v (cols 32v:32v+32) owns mag-bank rows 2v:2v+2: within the
    # variant, col 2v+r contracts w2 against the r-th 64-partition half of h
    w2v = np.zeros((128, GRP * 32), np.float16)
    for v in range(GRP):
        w2v[:H_DIM, 32 * v + 2 * v] = w2[:, 0]
        w2v[H_DIM:, 32 * v + 2 * v + 1] = w2[:, 0]

    # shard edges contiguously across cores; per-core transposed fp16 panel
    in_maps = []
    for c in range(N_CORES):
        sl = slice(c * EC, (c + 1) * EC)
        panel = np.zeros((128, ECP), np.float16)
        panel[:, :EC] = features[sl].T
        xt = np.zeros((NTILES, 128, XT_TILE), np.float16)
        a = 0
        for ti, size in enumerate(TILE_SIZES):
            w = size * SC
            xt[ti, :, :w] = panel[:, a:a + w]
            a += w
        in_maps.append({"xt": xt, "w1a": w1a, "w1b": w1b, "b1s": b1s, "w2v": w2v})

    nc = _get_nc()
    try:
        res = run_bass_kernel_spmd(nc, in_maps, core_ids=list(range(N_CORES)))
    except Exception:
        # one retry for transient runtime failures
        import time
        time.sleep(2.0)
        res = run_bass_kernel_spmd(nc, in_maps, core_ids=list(range(N_CORES)))
    LAST_RESULTS = res

    # decode mag: out [NMG, 32, 512]; flat index (G*16 + j)*1024 + r*512 + c
    # equals the edge index directly, so decode is a flat reshape
    mag = np.empty(E_TOT, np.float32)
    for c in range(N_CORES):
        arr = res.results[c]["mag"]  # [NMG, 32, 512]
        mag[c * EC:(c + 1) * EC] = arr.reshape(-1)[:EC]

    # fold b2 and the shifted-softplus constant: h_ref = h_dev - log(2)
    mag = mag + (b2[0] - np.float32(np.log(2.0)) * w2.sum())

    center = edge_index[0].astype(np.int64)
    neigh = edge_index[1].astype(np.int64)

    # scatter-mean debias per center atom
    cnt = np.bincount(center, minlength=N_ATOMS).astype(np.float32)
    ssum = np.bincount(center, weights=mag.astype(np.float64), minlength=N_ATOMS)
    bias = (ssum / np.maximum(cnt, 1.0)).astype(np.float32)
    mag = mag - bias[center]

    # pair-averaged antisymmetric force assembly (see module docstring)
    unit = edge_vectors / edge_lengths[:, None]
    val = (0.5 * mag)[:, None] * unit  # [E, 3]
    forces = np.zeros((N_ATOMS, 3), np.float32)
    for k in range(3):
        fc = np.bincount(center, weights=val[:, k].astype(np.float64), minlength=N_ATOMS)
        fn = np.bincount(neigh, weights=val[:, k].astype(np.float64), minlength=N_ATOMS)
        forces[:, k] = (fc - fn).astype(np.float32)
    return forces


# revision 37
# speedup vs baseline: 1.6423x; 1.0672x over previous
"""Trainium2 kernel for nn_DirectForce (gnn_message_passing).

Math (see reference):
    h   = softplus(X @ w1 + b1) - log(2)          per-edge MLP        [E, 64]
    mag = h @ w2 + b2                                                  [E, 1]
    mag = mag - mean_over_center(mag)[center]      scatter-mean debias
    pair-average mag between each directed edge and its reverse edge
    F   = segment_sum(mag * unit_vec, center)                          [N, 3]

The pair keys (center+neigh+length+|unit|) are identical exactly for the two
directions of each undirected edge (reverse edge has negated vector, same
length), so the sorted-pair averaging pairs each edge with its reverse.  Since
unit_rev = -unit, the pair-averaged scatter reduces algebraically to
    F = segsum(0.5*mag*unit, center) - segsum(0.5*mag*unit, neigh)
which removes the argsort entirely (verified to 2.5e-8 vs the reference).

Device (8 NeuronCores, SPMD, edges partitioned contiguously 200k/core):
  - features pre-transposed, cast to fp16, and pre-tiled on host to
    [NTILES, 128, XT_TILE] per core so every input DMA tile is contiguous in
    HBM; fp16 halves the dominant HBM traffic (819MB -> 410MB total).
  - mm1 in fp16: two matmuls per superchunk with zero-padded weights [w1;0]
    and [0;w1] accumulate stacked z = [zA; zB] [128, 512] f32 in PSUM.
    fp16 weights use the separate-LDWEIGHTS path, which the PE overlaps with
    in-flight matmuls via the background weight buffer (f32r could not:
    self-loading matmul serializes LDW+MM, which was the old 367us wall).
  - softplus: two ACT passes (this act_info has no Softplus table): Exp
    (bias=b1) per [128, 1024] z pair (two adjacent PSUM banks) into an fp16
    e tile, then ONE wide Ln(x+1) per 8-superchunk group over [128, 4096].
    Wide ops amortize the ~352-cycle ACT per-op overhead (old: 270us busy,
    new: ~0.99 ns/edge ~= 200us).  ACT is the remaining bottleneck; exp/ln
    have no single-table replacement and no other engine can evaluate them.
  - mm2: per h half, one matmul with a [128, 16] fp16 block-diag w2 variant
    (nonzero cols 2v:2v+2) accumulating into a shared [16, 512] f32 PSUM
    bank across 8 superchunks; rows it doesn't own get +0.  The full bank
    takes ONE batched DVE copy + DMA per 8192 edges (kills the old 149us of
    per-pair DVE copies; DMA itself has no PSUM port on TRN2), and the row
    layout is chosen so host decode is a single flat reshape.
Host post (index-structured tail, ~6% of input bytes, numpy):
  - debias via bincount, unit vectors, the two segment sums above.
"""

import numpy as np

N_ATOMS = 50000
E_TOT = 1600000
D_FEAT = 128
H_DIM = 64
N_CORES = 8
EC = E_TOT // N_CORES          # 200000 edges per core
SC = 1024                      # edges per superchunk (2 chunks of 512)
NSC = (EC + SC - 1) // SC      # 196 superchunks
ECP = NSC * SC                 # 200704 padded edges per core
XT_TILE = 8192                 # edges per input DMA tile
GRP = 16                       # superchunks per mag PSUM bank (32 rows)
NMG = (NSC + GRP - 1) // GRP   # 13 mag banks (last one quarter-filled)

# input-tile taper: small tiles at the head (compute starts fast) and tail
# (short pipeline drain), big 8-superchunk tiles between
TILE_SIZES = [1, 1, 2, 4] + [8] * 22 + [4, 2, 2, 2, 1, 1]
assert sum(TILE_SIZES) == NSC
NTILES = len(TILE_SIZES)

_CACHE = {}
LAST_RESULTS = None


def _patch_act_tables():
    """Make Exp and Ln resolve to the single table set that contains both
    (natural_log_exp_and_others) so the ACT table is loaded exactly once;
    the default per-op greedy choice flip-flops between exp_and_friends and
    natural_log, paying ~1.5us per reload.  Table-set ids are positional,
    so keys/order are preserved."""
    import functools
    import concourse.hw_specs as hw_specs
    import concourse.bacc as bacc_mod
    import concourse.mybir as mybir

    if _CACHE.get("tables_patched"):
        return
    orig = hw_specs.get_activation_tables
    Exp = mybir.ActivationFunctionType.Exp
    Ln = mybir.ActivationFunctionType.Ln

    def patched(arch):
        out = {}
        for name, fns in orig(arch).items():
            if name != "natural_log_exp_and_others":
                fns = fns - {Exp, Ln}
            out[name] = fns
        return out

    cached = functools.cache(patched)
    hw_specs.get_activation_tables = cached
    bacc_mod.get_activation_tables = cached
    _CACHE["tables_patched"] = True


def _build_nc():
    import concourse.bacc as bacc
    import concourse.mybir as mybir
    import concourse.tile as tile

    _patch_act_tables()

    F32 = mybir.dt.float32
    F16 = mybir.dt.float16
    Exp = mybir.ActivationFunctionType.Exp
    Ln = mybir.ActivationFunctionType.Ln

    nc = bacc.Bacc("TRN2", target_bir_lowering=False, debug=False)
    xt_d = nc.dram_tensor("xt", [NTILES, 128, XT_TILE], F16, kind="ExternalInput")
    w1a_d = nc.dram_tensor("w1a", [128, 128], F16, kind="ExternalInput")
    w1b_d = nc.dram_tensor("w1b", [128, 128], F16, kind="ExternalInput")
    b1_d = nc.dram_tensor("b1s", [128, 1], F32, kind="ExternalInput")
    w2v_d = nc.dram_tensor("w2v", [128, GRP * 32], F16, kind="ExternalInput")
    mag_d = nc.dram_tensor("mag", [NMG, 32, 512], F32, kind="ExternalOutput")

    with tile.TileContext(nc) as tc:
        with (
            tc.tile_pool(name="wp", bufs=1) as wp,
            tc.tile_pool(name="xp", bufs=4) as xp,
            tc.tile_pool(name="ep", bufs=2) as ep,
            tc.tile_pool(name="hp", bufs=2) as hp,
            tc.tile_pool(name="mp", bufs=3) as mp,
            tc.tile_pool(name="zp", bufs=3, space="PSUM") as zp,
            tc.tile_pool(name="magp", bufs=2, space="PSUM") as magp,
        ):
            # dummy early activation: hoists the ~1.5us ACT table load off
            # the first real activation's dependency chain
            dummy = wp.tile([128, 1], F32, tag="dummy")
            nc.vector.memset(dummy[:], 0.0)
            nc.scalar.activation(dummy[:], dummy[:], Exp, bias=0.0)

            w1a = wp.tile([128, 128], F16, tag="w1a")
            w1b = wp.tile([128, 128], F16, tag="w1b")
            b1s = wp.tile([128, 1], F32, tag="b1s")
            w2v = wp.tile([128, GRP * 32], F16, tag="w2v")
            nc.gpsimd.dma_start(w1a[:], w1a_d[:])
            nc.gpsimd.dma_start(w1b[:], w1b_d[:])
            nc.gpsimd.dma_start(b1s[:], b1_d[:])
            nc.gpsimd.dma_start(w2v[:], w2v_d[:])

            z_ps = None
            e_sb = None
            h_state = [None]
            MM2_DELAY = 2      # pairs between Ln-half and mm2 eligibility
            pending_mm2 = []   # [(pair#, h tile, v, stop_bank, grp idx)]

            mag_state = [None]

            def trickle_mm2(pair_no, limit):
                # emit up to `limit` single mm2 matmuls whose Ln-half was
                # issued >= MM2_DELAY pairs ago.  Spreading mm2s keeps the
                # PE instruction stream uniform (no 8-deep mm2 bursts that
                # block later mm1s in the strict FIFO and starve ACT) and
                # avoids the HAM re-throttle from bursty PE idling.
                n = 0
                while pending_mm2 and n < limit and (
                    pair_no is None
                    or pair_no - pending_mm2[0][0] >= MM2_DELAY
                ):
                    _, h_sb, v, stop_bank, gi = pending_mm2.pop(0)
                    if v == 0:
                        mag_new = magp.tile([32, 512], F32, tag="mag")
                        mag_state[0] = mag_new
                    mag_ps = mag_state[0]
                    nc.tensor.matmul(
                        mag_ps[:], w2v[:, 32 * v:32 * (v + 1)],
                        h_sb[:, v * 512:(v + 1) * 512],
                        start=(v == 0), stop=stop_bank,
                        skip_group_check=True,
                    )
                    if stop_bank:
                        # bank complete: DMA has no PSUM port on TRN2 ->
                        # one batched DVE copy per bank, then DMA from SBUF
                        mag_sb = mp.tile([32, 512], F32, tag="magsb")
                        nc.vector.tensor_copy(mag_sb[:], mag_ps[:])
                        nc.gpsimd.dma_start(mag_d[gi], mag_sb[:])
                    n += 1

            g = 0
            for ti, size in enumerate(TILE_SIZES):
                width = size * SC
                xt = xp.tile([128, XT_TILE], F16, tag="xt")
                nc.sync.dma_start(xt[:, :width], xt_d[ti, :, :width])
                for t in range(size):
                    off = t * SC
                    j = g % GRP          # superchunk slot within the group
                    ei = g % 2           # slot within the z pair tile
                    if ei == 0:
                        z_ps = zp.tile([128, 1024], F32, tag="z")
                    zs = z_ps[:, ei * 512:(ei + 1) * 512]
                    nc.tensor.matmul(
                        zs, w1a[:], xt[:, off:off + 512],
                        start=True, stop=False,
                    )
                    nc.tensor.matmul(
                        zs, w1b[:], xt[:, off + 512:off + 1024],
                        start=False, stop=True,
                    )
                    if ei == 1:
                        trickle_mm2(g // 2, 2)
                        q = (j - 1) // 2     # pair slot within the group
                        if q == 0:
                            e_sb = ep.tile([128, GRP * 512], F16, tag="e")
                        nc.scalar.activation(
                            e_sb[:, q * 1024:(q + 1) * 1024], z_ps[:],
                            Exp, bias=b1s[:, :1],
                        )
                        half = GRP // 4   # pairs per Ln half-op
                        if (q + 1) % half == 0 or g + 1 == NSC:
                            q0 = (q // half) * half   # first pair this half
                            if q0 == 0:
                                h_new = hp.tile([128, GRP * 512], F16, tag="h")
                                h_state[0] = h_new
                            hcur = h_state[0]
                            nc.scalar.activation(
                                hcur[:, q0 * 1024:(q + 1) * 1024],
                                e_sb[:, q0 * 1024:(q + 1) * 1024],
                                Ln, bias=1.0,
                            )
                            bank_done = (j == GRP - 1) or (g + 1 == NSC)
                            v1 = 2 * (q + 1)
                            for v in range(2 * q0, v1):
                                pending_mm2.append(
                                    (g // 2, hcur, v,
                                     bank_done and v == v1 - 1, g // GRP)
                                )
                    g += 1
            trickle_mm2(None, len(pending_mm2))
    nc.compile()
    return nc


def _get_nc():
    if "nc" not in _CACHE:
        _CACHE["nc"] = _build_nc()
    return _CACHE["nc"]


def kernel(features, edge_vectors, edge_lengths, edge_index, w1, b1, w2, b2):
    global LAST_RESULTS
    from concourse.bass_utils import run_bass_kernel_spmd

    features = np.asarray(features, dtype=np.float32)
    edge_vectors = np.asarray(edge_vectors, dtype=np.float32)
    edge_lengths = np.asarray(edge_lengths, dtype=np.float32)
    edge_index = np.asarray(edge_index)
    w1 = np.asarray(w1, dtype=np.float32)
    b1 = np.asarray(b1, dtype=np.float32).reshape(-1)
    w2 = np.asarray(w2, dtype=np.float32).reshape(-1, 1)
    b2 = np.asarray(b2, dtype=np.float32).reshape(-1)

    # replicated small weights, padded for the stacked-z / block-diag tricks
    w1a = np.zeros((128, 128), np.float16)
    w1a[:, :H_DIM] = w1
    w1b = np.zeros((128, 128), np.float16)
    w1b[:, H_DIM:] = w1
    b1s = np.concatenate([b1, b1]).astype(np.float32).reshape(128, 1)
    # mm2 variant v (cols 32v:32v+32) owns mag-bank rows 2v:2v+2: within the
    # variant, col 2v+r contracts w2 against the r-th 64-partition half of h
    w2v = np.zeros((128, GRP * 32), np.float16)
    for v in range(GRP):
        w2v[:H_DIM, 32 * v + 2 * v] = w2[:, 0]
        w2v[H_DIM:, 32 * v + 2 * v + 1] = w2[:, 0]

    # shard edges contiguously across cores; per-core transposed fp16 panel
    in_maps = []
    for c in range(N_CORES):
        sl = slice(c * EC, (c + 1) * EC)
        panel = np.zeros((128, ECP), np.float16)
        panel[:, :EC] = features[sl].T
        xt = np.zeros((NTILES, 128, XT_TILE), np.float16)
        a = 0
        for ti, size in enumerate(TILE_SIZES):
            w = size * SC
            xt[ti, :, :w] = panel[:, a:a + w]
            a += w
        in_maps.append({"xt": xt, "w1a": w1a, "w1b": w1b, "b1s": b1s, "w2v": w2v})

    nc = _get_nc()
    try:
        res = run_bass_kernel_spmd(nc, in_maps, core_ids=list(range(N_CORES)))
    except Exception:
        # one retry for transient runtime failures
        import time
        time.sleep(2.0)
        res = run_bass_kernel_spmd(nc, in_maps, core_ids=list(range(N_CORES)))
    LAST_RESULTS = res

    # decode mag: out [NMG, 32, 512]; flat index (G*16 + j)*1024 + r*512 + c
    # equals the edge index directly, so decode is a flat reshape
    mag = np.empty(E_TOT, np.float32)
    for c in range(N_CORES):
        arr = res.results[c]["mag"]  # [NMG, 32, 512]
        mag[c * EC:(c + 1) * EC] = arr.reshape(-1)[:EC]

    # fold b2 and the shifted-softplus constant: h_ref = h_dev - log(2)
    mag = mag + (b2[0] - np.float32(np.log(2.0)) * w2.sum())

    center = edge_index[0].astype(np.int64)
    neigh = edge_index[1].astype(np.int64)

    # scatter-mean debias per center atom
    cnt = np.bincount(center, minlength=N_ATOMS).astype(np.float32)
    ssum = np.bincount(center, weights=mag.astype(np.float64), minlength=N_ATOMS)
    bias = (ssum / np.maximum(cnt, 1.0)).astype(np.float32)
    mag = mag - bias[center]

    # pair-averaged antisymmetric force assembly (see module docstring)
    unit = edge_vectors / edge_lengths[:, None]
    val = (0.5 * mag)[:, None] * unit  # [E, 3]
    forces = np.zeros((N_ATOMS, 3), np.float32)
    for k in range(3):
        fc = np.bincount(center, weights=val[:, k].astype(np.float64), minlength=N_ATOMS)
        fn = np.bincount(neigh, weights=val[:, k].astype(np.float64), minlength=N_ATOMS)
        forces[:, k] = (fc - fn).astype(np.float32)
    return forces


# revision 39
# speedup vs baseline: 1.6474x; 1.0031x over previous
"""Trainium2 kernel for nn_DirectForce (gnn_message_passing).

Math (see reference):
    h   = softplus(X @ w1 + b1) - log(2)          per-edge MLP        [E, 64]
    mag = h @ w2 + b2                                                  [E, 1]
    mag = mag - mean_over_center(mag)[center]      scatter-mean debias
    pair-average mag between each directed edge and its reverse edge
    F   = segment_sum(mag * unit_vec, center)                          [N, 3]

The pair keys (center+neigh+length+|unit|) are identical exactly for the two
directions of each undirected edge (reverse edge has negated vector, same
length), so the sorted-pair averaging pairs each edge with its reverse.  Since
unit_rev = -unit, the pair-averaged scatter reduces algebraically to
    F = segsum(0.5*mag*unit, center) - segsum(0.5*mag*unit, neigh)
which removes the argsort entirely (verified to 2.5e-8 vs the reference).

Device (8 NeuronCores, SPMD, edges partitioned contiguously 200k/core):
  - features pre-transposed, cast to fp16, and pre-tiled on host to
    [NTILES, 128, XT_TILE] per core so every input DMA tile is contiguous in
    HBM; fp16 halves the dominant HBM traffic (819MB -> 410MB total).
  - mm1 in fp16: two matmuls per superchunk with zero-padded weights [w1;0]
    and [0;w1] accumulate stacked z = [zA; zB] [128, 512] f32 in PSUM.
    fp16 weights use the separate-LDWEIGHTS path, which the PE overlaps with
    in-flight matmuls via the background weight buffer (f32r could not:
    self-loading matmul serializes LDW+MM, which was the old 367us wall).
  - softplus: two ACT passes (this act_info has no Softplus table): Exp
    (bias=b1) per [128, 1024] z pair (two adjacent PSUM banks) into an fp16
    e tile, then ONE wide Ln(x+1) per 8-superchunk group over [128, 4096].
    Wide ops amortize the ~352-cycle ACT per-op overhead (old: 270us busy,
    new: ~0.99 ns/edge ~= 200us).  ACT is the remaining bottleneck; exp/ln
    have no single-table replacement and no other engine can evaluate them.
  - mm2: per h half, one matmul with a [128, 16] fp16 block-diag w2 variant
    (nonzero cols 2v:2v+2) accumulating into a shared [16, 512] f32 PSUM
    bank across 8 superchunks; rows it doesn't own get +0.  The full bank
    takes ONE batched DVE copy + DMA per 8192 edges (kills the old 149us of
    per-pair DVE copies; DMA itself has no PSUM port on TRN2), and the row
    layout is chosen so host decode is a single flat reshape.
Host post (index-structured tail, ~6% of input bytes, numpy):
  - debias via bincount, unit vectors, the two segment sums above.
"""

import numpy as np

N_ATOMS = 50000
E_TOT = 1600000
D_FEAT = 128
H_DIM = 64
N_CORES = 8
EC = E_TOT // N_CORES          # 200000 edges per core
SC = 1024                      # edges per superchunk (2 chunks of 512)
NSC = (EC + SC - 1) // SC      # 196 superchunks
ECP = NSC * SC                 # 200704 padded edges per core
XT_TILE = 8192                 # edges per input DMA tile
GRP = 16                       # superchunks per mag PSUM bank (32 rows)
NMG = (NSC + GRP - 1) // GRP   # 13 mag banks (last one quarter-filled)

# input-tile taper: small tiles at the head (compute starts fast; the first
# tile covers a full superchunk *pair* so one DMA unblocks the first Exp)
# and tail (short pipeline drain), big 8-superchunk tiles between
TILE_SIZES = [2, 2, 4] + [8] * 22 + [4, 2, 2, 2, 1, 1]
assert sum(TILE_SIZES) == NSC
NTILES = len(TILE_SIZES)

_CACHE = {}
LAST_RESULTS = None


def _patch_act_tables():
    """Make Exp and Ln resolve to the single table set that contains both
    (natural_log_exp_and_others) so the ACT table is loaded exactly once;
    the default per-op greedy choice flip-flops between exp_and_friends and
    natural_log, paying ~1.5us per reload.  Table-set ids are positional,
    so keys/order are preserved."""
    import functools
    import concourse.hw_specs as hw_specs
    import concourse.bacc as bacc_mod
    import concourse.mybir as mybir

    if _CACHE.get("tables_patched"):
        return
    orig = hw_specs.get_activation_tables
    Exp = mybir.ActivationFunctionType.Exp
    Ln = mybir.ActivationFunctionType.Ln

    def patched(arch):
        out = {}
        for name, fns in orig(arch).items():
            if name != "natural_log_exp_and_others":
                fns = fns - {Exp, Ln}
            out[name] = fns
        return out

    cached = functools.cache(patched)
    hw_specs.get_activation_tables = cached
    bacc_mod.get_activation_tables = cached
    _CACHE["tables_patched"] = True


def _build_nc():
    import concourse.bacc as bacc
    import concourse.mybir as mybir
    import concourse.tile as tile

    _patch_act_tables()

    F32 = mybir.dt.float32
    F16 = mybir.dt.float16
    Exp = mybir.ActivationFunctionType.Exp
    Ln = mybir.ActivationFunctionType.Ln

    nc = bacc.Bacc("TRN2", target_bir_lowering=False, debug=False)
    xt_d = nc.dram_tensor("xt", [NTILES, 128, XT_TILE], F16, kind="ExternalInput")
    w1a_d = nc.dram_tensor("w1a", [128, 128], F16, kind="ExternalInput")
    w1b_d = nc.dram_tensor("w1b", [128, 128], F16, kind="ExternalInput")
    b1_d = nc.dram_tensor("b1s", [128, 1], F32, kind="ExternalInput")
    w2v_d = nc.dram_tensor("w2v", [128, GRP * 32], F16, kind="ExternalInput")
    mag_d = nc.dram_tensor("mag", [NMG, 32, 512], F32, kind="ExternalOutput")

    with tile.TileContext(nc) as tc:
        with (
            tc.tile_pool(name="wp", bufs=1) as wp,
            tc.tile_pool(name="xp", bufs=4) as xp,
            tc.tile_pool(name="ep", bufs=2) as ep,
            tc.tile_pool(name="hp", bufs=2) as hp,
            tc.tile_pool(name="mp", bufs=3) as mp,
            tc.tile_pool(name="zp", bufs=3, space="PSUM") as zp,
            tc.tile_pool(name="magp", bufs=2, space="PSUM") as magp,
        ):
            # dummy early activation: hoists the ~1.5us ACT table load off
            # the first real activation's dependency chain
            dummy = wp.tile([128, 1], F32, tag="dummy")
            nc.vector.memset(dummy[:], 0.0)
            nc.scalar.activation(dummy[:], dummy[:], Exp, bias=0.0)

            w1a = wp.tile([128, 128], F16, tag="w1a")
            w1b = wp.tile([128, 128], F16, tag="w1b")
            b1s = wp.tile([128, 1], F32, tag="b1s")
            w2v = wp.tile([128, GRP * 32], F16, tag="w2v")
            nc.gpsimd.dma_start(w1a[:], w1a_d[:])
            nc.gpsimd.dma_start(w1b[:], w1b_d[:])
            # b1s on the scalar HWDGE queue: it gates the first Exp and must
            # not wait behind the weight transfers on the gpsimd SWDGE queue
            nc.scalar.dma_start(b1s[:], b1_d[:])
            nc.gpsimd.dma_start(w2v[:], w2v_d[:])

            z_ps = None
            e_sb = None
            h_state = [None]
            MM2_DELAY = 2      # pairs between Ln-half and mm2 eligibility
            pending_mm2 = []   # [(pair#, h tile, v, stop_bank, grp idx)]

            mag_state = [None]

            def trickle_mm2(pair_no, limit):
                # emit up to `limit` single mm2 matmuls whose Ln-half was
                # issued >= MM2_DELAY pairs ago.  Spreading mm2s keeps the
                # PE instruction stream uniform (no 8-deep mm2 bursts that
                # block later mm1s in the strict FIFO and starve ACT) and
                # avoids the HAM re-throttle from bursty PE idling.
                n = 0
                while pending_mm2 and n < limit and (
                    pair_no is None
                    or pair_no - pending_mm2[0][0] >= MM2_DELAY
                ):
                    _, h_sb, v, stop_bank, gi = pending_mm2.pop(0)
                    if v == 0:
                        mag_new = magp.tile([32, 512], F32, tag="mag")
                        mag_state[0] = mag_new
                    mag_ps = mag_state[0]
                    nc.tensor.matmul(
                        mag_ps[:], w2v[:, 32 * v:32 * (v + 1)],
                        h_sb[:, v * 512:(v + 1) * 512],
                        start=(v == 0), stop=stop_bank,
                        skip_group_check=True,
                    )
                    if stop_bank:
                        # bank complete: DMA has no PSUM port on TRN2 ->
                        # one batched DVE copy per bank, then DMA from SBUF
                        mag_sb = mp.tile([32, 512], F32, tag="magsb")
                        nc.vector.tensor_copy(mag_sb[:], mag_ps[:])
                        nc.gpsimd.dma_start(mag_d[gi], mag_sb[:])
                    n += 1

            g = 0
            for ti, size in enumerate(TILE_SIZES):
                width = size * SC
                xt = xp.tile([128, XT_TILE], F16, tag="xt")
                nc.sync.dma_start(xt[:, :width], xt_d[ti, :, :width])
                for t in range(size):
                    off = t * SC
                    j = g % GRP          # superchunk slot within the group
                    ei = g % 2           # slot within the z pair tile
                    if ei == 0:
                        z_ps = zp.tile([128, 1024], F32, tag="z")
                    zs = z_ps[:, ei * 512:(ei + 1) * 512]
                    nc.tensor.matmul(
                        zs, w1a[:], xt[:, off:off + 512],
                        start=True, stop=False,
                    )
                    nc.tensor.matmul(
                        zs, w1b[:], xt[:, off + 512:off + 1024],
                        start=False, stop=True,
                    )
                    if ei == 1:
                        trickle_mm2(g // 2, 2)
                        q = (j - 1) // 2     # pair slot within the group
                        if q == 0:
                            e_sb = ep.tile([128, GRP * 512], F16, tag="e")
                        nc.scalar.activation(
                            e_sb[:, q * 1024:(q + 1) * 1024], z_ps[:],
                            Exp, bias=b1s[:, :1],
                        )
                        half = GRP // 4   # pairs per Ln half-op
                        if (q + 1) % half == 0 or g + 1 == NSC:
                            q0 = (q // half) * half   # first pair this half
                            if q0 == 0:
                                h_new = hp.tile([128, GRP * 512], F16, tag="h")
                                h_state[0] = h_new
                            hcur = h_state[0]
                            nc.scalar.activation(
                                hcur[:, q0 * 1024:(q + 1) * 1024],
                                e_sb[:, q0 * 1024:(q + 1) * 1024],
                                Ln, bias=1.0,
                            )
                            bank_done = (j == GRP - 1) or (g + 1 == NSC)
                            v1 = 2 * (q + 1)
                            for v in range(2 * q0, v1):
                                pending_mm2.append(
                                    (g // 2, hcur, v,
                                     bank_done and v == v1 - 1, g // GRP)
                                )
                    g += 1
            trickle_mm2(None, len(pending_mm2))
    nc.compile()
    return nc


def _get_nc():
    if "nc" not in _CACHE:
        _CACHE["nc"] = _build_nc()
    return _CACHE["nc"]


def kernel(features, edge_vectors, edge_lengths, edge_index, w1, b1, w2, b2):
    global LAST_RESULTS
    from concourse.bass_utils import run_bass_kernel_spmd

    features = np.asarray(features, dtype=np.float32)
    edge_vectors = np.asarray(edge_vectors, dtype=np.float32)
    edge_lengths = np.asarray(edge_lengths, dtype=np.float32)
    edge_index = np.asarray(edge_index)
    w1 = np.asarray(w1, dtype=np.float32)
    b1 = np.asarray(b1, dtype=np.float32).reshape(-1)
    w2 = np.asarray(w2, dtype=np.float32).reshape(-1, 1)
    b2 = np.asarray(b2, dtype=np.float32).reshape(-1)

    # replicated small weights, padded for the stacked-z / block-diag tricks
    w1a = np.zeros((128, 128), np.float16)
    w1a[:, :H_DIM] = w1
    w1b = np.zeros((128, 128), np.float16)
    w1b[:, H_DIM:] = w1
    b1s = np.concatenate([b1, b1]).astype(np.float32).reshape(128, 1)
    # mm2 variant v (cols 32v:32v+32) owns mag-bank rows 2v:2v+2: within the
    # variant, col 2v+r contracts w2 against the r-th 64-partition half of h
    w2v = np.zeros((128, GRP * 32), np.float16)
    for v in range(GRP):
        w2v[:H_DIM, 32 * v + 2 * v] = w2[:, 0]
        w2v[H_DIM:, 32 * v + 2 * v + 1] = w2[:, 0]

    # shard edges contiguously across cores; per-core transposed fp16 panel
    in_maps = []
    for c in range(N_CORES):
        sl = slice(c * EC, (c + 1) * EC)
        panel = np.zeros((128, ECP), np.float16)
        panel[:, :EC] = features[sl].T
        xt = np.zeros((NTILES, 128, XT_TILE), np.float16)
        a = 0
        for ti, size in enumerate(TILE_SIZES):
            w = size * SC
            xt[ti, :, :w] = panel[:, a:a + w]
            a += w
        in_maps.append({"xt": xt, "w1a": w1a, "w1b": w1b, "b1s": b1s, "w2v": w2v})

    nc = _get_nc()
    try:
        res = run_bass_kernel_spmd(nc, in_maps, core_ids=list(range(N_CORES)))
    except Exception:
        # one retry for transient runtime failures
        import time
        time.sleep(2.0)
        res = run_bass_kernel_spmd(nc, in_maps, core_ids=list(range(N_CORES)))
    LAST_RESULTS = res

    # decode mag: out [NMG, 32, 512]; flat index (G*16 + j)*1024 + r*512 + c
    # equals the edge index directly, so decode is a flat reshape
    mag = np.empty(E_TOT, np.float32)
    for c in range(N_CORES):
        arr = res.results[c]["mag"]  # [NMG, 32, 512]
        mag[c * EC:(c + 1) * EC] = arr.reshape(-1)[:EC]

    # fold b2 and the shifted-softplus constant: h_ref = h_dev - log(2)
    mag = mag + (b2[0] - np.float32(np.log(2.0)) * w2.sum())

    center = edge_index[0].astype(np.int64)
    neigh = edge_index[1].astype(np.int64)

    # scatter-mean debias per center atom
    cnt = np.bincount(center, minlength=N_ATOMS).astype(np.float32)
    ssum = np.bincount(center, weights=mag.astype(np.float64), minlength=N_ATOMS)
    bias = (ssum / np.maximum(cnt, 1.0)).astype(np.float32)
    mag = mag - bias[center]

    # pair-averaged antisymmetric force assembly (see module docstring)
    unit = edge_vectors / edge_lengths[:, None]
    val = (0.5 * mag)[:, None] * unit  # [E, 3]
    forces = np.zeros((N_ATOMS, 3), np.float32)
    for k in range(3):
        fc = np.bincount(center, weights=val[:, k].astype(np.float64), minlength=N_ATOMS)
        fn = np.bincount(neigh, weights=val[:, k].astype(np.float64), minlength=N_ATOMS)
        forces[:, k] = (fc - fn).astype(np.float32)
    return forces
